# revision 9
# baseline (speedup 1.0000x reference)
"""Trainium2 Bass kernel for nn_Attention_17532056502607.

Multi-head self-attention (B=8, N=48*48=2304 tokens, C=384, 8 heads of 48):
    q = x @ q_w.T + q_b ; k,v = x @ kv_w.T + kv_b
    out = softmax(q k^T / sqrt(48)) v ; y = out @ proj_w.T + proj_b

Sharding: data-parallel, one batch element per NeuronCore (8 cores).

Per-core algorithm (all in "S^T layout", keys on partitions — no transposes):
  - host supplies xT = x_b^T [C, N] and head-PAIR-packed weights: heads 2p /
    2p+1 of a pair live at partition rows 0-47 / 64-111, so two K=48 matmuls
    run concurrently in the PE array (row/col 32-tiles).
  - qT/kT [C_pair, N] = wT-pair @ xT          (PE, K=C=384)
  - v    [N, 8*49]    = xT-pair-cols... v natural + a ones column per head
    (ones column injected via a K=1 rank-1 matmul with the v-bias row), so
    attn@V also accumulates the softmax denominator.
  - S^T  [keys, q]    = kT-tile.T @ qT        (K=48, row-packed head pairs)
  - expS = exp(scale * S^T)                   (ACT, reads PSUM, writes SBUF)
  - outT [49*2, q]   += (v|1).T @ expS        (K=128 keys, col-packed pairs)
    row 48/112 of outT = softmax denominators.
  - normalize: recip(denoms) -> rank-1 ones-matmul broadcast -> DVE multiply
  - y    [N, C]       = sum_pairs outT-pair.T @ projw-pair + bias (row-packed)
"""

import os
import sys

import numpy as np

for _p in ("/opt/trn_rl_repo",):
    if _p not in sys.path:
        sys.path.append(_p)

import concourse.bass as bass  # noqa: E402
import concourse.tile as tile  # noqa: E402
from concourse import bacc, mybir  # noqa: E402
from concourse.bass_utils import run_bass_kernel_spmd  # noqa: E402

# ---------------------------------------------------------------- constants
B = 8
HH = 48
WW = 48
C = 384
N = HH * WW  # 2304
NH = 8
HD = 48
PAIRS = NH // 2  # 4
P = 128
NT = N // P  # 18 token tiles
KTC = C // P  # 3 contraction tiles over C
SCALE = float(HD) ** -0.5
VW = NH * (HD + 1)  # 392: v with a ones column per head
CHUNKS = [(0, 512), (512, 512), (1024, 512), (1536, 512), (2048, 256)]

F32 = mybir.dt.float32
# Matmul dtype for all SBUF operands. float32 = exact but 4 cyc/row on PE;
# float32r = same bits, reduced-precision single-pass matmul (1 cyc/row for
# moving dim >= 256); bfloat16 halves SBUF/DMA and enables FWL.
MM_DT = getattr(mybir.dt, os.environ.get("ATTN_MM_DT", "float32r"))

_EXP = mybir.ActivationFunctionType.Exp


def _emit(tc: tile.TileContext, d: dict, ctx):
    nc = tc.nc

    persist = ctx.enter_context(tc.tile_pool(name="persist", bufs=1))
    v_sb = persist.tile([P, NT, VW], MM_DT, name="v_sb")
    qT_sb = persist.tile([P, PAIRS, N], MM_DT, name="qT_sb")
    kT_sb = persist.tile([P, PAIRS, N], MM_DT, name="kT_sb")
    oT_sb = persist.tile([P, PAIRS, N], MM_DT, name="oT_sb")
    pw_sb = persist.tile([P, PAIRS, C], MM_DT, name="pw_sb")
    qb_sb = persist.tile([P, PAIRS], F32, name="qb_sb")
    kb_sb = persist.tile([P, PAIRS], F32, name="kb_sb")
    vb_sb = persist.tile([1, VW], MM_DT, name="vb_sb")
    pb_sb = persist.tile([1, C], MM_DT, name="pb_sb")
    ones_sb = persist.tile([1, P], MM_DT, name="ones_sb")

    nc.sync.dma_start(pw_sb[:], d["pwP"].rearrange("r p m -> p r m"))
    nc.sync.dma_start(qb_sb[:], d["qbP"])
    nc.sync.dma_start(kb_sb[:], d["kbP"])
    nc.sync.dma_start(vb_sb[:], d["vbA"])
    nc.sync.dma_start(pb_sb[:], d["pbR"])
    nc.vector.memset(ones_sb[:], 1.0)

    # ---------------- phase A: projections q^T, k^T, v ----------------
    with (
        tc.tile_pool(name="phA", bufs=1) as phA,
        tc.tile_pool(name="psA", bufs=4, space="PSUM") as psA,
    ):
        xT_sb = phA.tile([P, KTC, N], MM_DT, name="xT_sb")
        wq_sb = phA.tile([P, KTC, PAIRS * P], MM_DT, name="wq_sb")
        wk_sb = phA.tile([P, KTC, PAIRS * P], MM_DT, name="wk_sb")
        wv_sb = phA.tile([P, KTC, VW], MM_DT, name="wv_sb")
        nc.sync.dma_start(xT_sb[:], d["xT"].rearrange("(kt p) n -> p kt n", p=P))
        nc.sync.dma_start(wq_sb[:], d["wqP"].rearrange("(kt p) m -> p kt m", p=P))
        nc.sync.dma_start(wk_sb[:], d["wkP"].rearrange("(kt p) m -> p kt m", p=P))
        nc.sync.dma_start(wv_sb[:], d["wvA"].rearrange("(kt p) m -> p kt m", p=P))

        # v natural [token, 8*(hd|1)]: K=C matmul + rank-1 (ones x vb_aug)
        # which adds the v bias AND writes 1.0 into each head's 49th column.
        for nt in range(NT):
            psv = psA.tile([P, 512], F32, name="psv", tag="psA")
            for kt in range(KTC):
                nc.tensor.matmul(
                    psv[:, 0:VW],
                    lhsT=xT_sb[:, kt, nt * P : (nt + 1) * P],
                    rhs=wv_sb[:, kt, :],
                    start=(kt == 0),
                    stop=False,
                )
            nc.tensor.matmul(
                psv[:, 0:VW],
                lhsT=ones_sb[:, 0:P],
                rhs=vb_sb[:],
                start=False,
                stop=True,
            )
            nc.vector.tensor_copy(v_sb[:, nt, :], psv[:, 0:VW])

        # q^T/k^T in pair layout: out partitions = pair block of C_out.
        for pr in range(PAIRS):
            for q0, qw in CHUNKS:
                psq = psA.tile([P, 512], F32, name="psq", tag="psA")
                psk = psA.tile([P, 512], F32, name="psk", tag="psA")
                for kt in range(KTC):
                    nc.tensor.matmul(
                        psq[:, 0:qw],
                        lhsT=wq_sb[:, kt, pr * P : (pr + 1) * P],
                        rhs=xT_sb[:, kt, q0 : q0 + qw],
                        start=(kt == 0),
                        stop=(kt == KTC - 1),
                    )
                    nc.tensor.matmul(
                        psk[:, 0:qw],
                        lhsT=wk_sb[:, kt, pr * P : (pr + 1) * P],
                        rhs=xT_sb[:, kt, q0 : q0 + qw],
                        start=(kt == 0),
                        stop=(kt == KTC - 1),
                    )
                nc.vector.tensor_scalar_add(
                    qT_sb[:, pr, q0 : q0 + qw], psq[:, 0:qw], qb_sb[:, pr : pr + 1]
                )
                nc.vector.tensor_scalar_add(
                    kT_sb[:, pr, q0 : q0 + qw], psk[:, 0:qw], kb_sb[:, pr : pr + 1]
                )

    # ---------------- attention: flash over q chunks, S^T layout -------
    with (
        tc.tile_pool(name="es", bufs=3) as es_pool,
        tc.tile_pool(name="rcp", bufs=4) as rc_pool,
        tc.tile_pool(name="psS", bufs=2, space="PSUM") as psS,
        tc.tile_pool(name="psO", bufs=1, space="PSUM") as psO,
    ):
        for pr in range(PAIRS):
            for q0, qw in CHUNKS:
                # separate accumulator banks per head; col strips 0-1 vs 2-3
                # (out base partition 0 vs 64) keep the two matmuls concurrent
                oTA = psO.tile([P, 512], F32, name="oTA", tag="oTA")
                oTB = psO.tile([P, 512], F32, name="oTB", tag="oTB")
                # slot sequence: si = 2*kt + head_bit, grouped 3 per psum tile
                seq = [(kt, hoff) for kt in range(NT) for hoff in (0, 64)]
                sg = None
                for si, (kt, hoff) in enumerate(seq):
                    g, j = divmod(si, 3)
                    if j == 0:
                        sg = psS.tile([P, 3, 512], F32, name="sg", tag="sg")
                    nc.tensor.matmul(
                        sg[:, j, 0:qw],
                        lhsT=kT_sb[hoff : hoff + HD, pr, kt * P : (kt + 1) * P],
                        rhs=qT_sb[hoff : hoff + HD, pr, q0 : q0 + qw],
                        start=True,
                        stop=True,
                    )
                    if j == 2:
                        est = es_pool.tile([P, 3, 512], MM_DT, name="est", tag="est")
                        nc.scalar.activation(
                            est[:, :, 0:qw], sg[:, :, 0:qw], _EXP, scale=SCALE
                        )
                        for jj in range(3):
                            kt2, hoff2 = seq[g * 3 + jj]
                            h = pr * 2 + (0 if hoff2 == 0 else 1)
                            oT = oTA if hoff2 == 0 else oTB
                            nc.tensor.matmul(
                                oT[hoff2 : hoff2 + HD + 1, 0:qw],
                                lhsT=v_sb[:, kt2, h * (HD + 1) : (h + 1) * (HD + 1)],
                                rhs=est[:, jj, 0:qw],
                                start=(kt2 == 0),
                                stop=(kt2 == NT - 1),
                            )
                # normalize: row 0 of oTA / row 64 of oTB = denominators
                # (ones column leads each V block so denoms land 32-aligned)
                rA = rc_pool.tile([1, 512], MM_DT, name="rA", tag="rc")
                rB = rc_pool.tile([1, 512], MM_DT, name="rB", tag="rc")
                nc.vector.reciprocal(rA[:, 0:qw], oTA[0:1, 0:qw])
                nc.vector.reciprocal(rB[:, 0:qw], oTB[64:65, 0:qw])
                bc = psS.tile([P, 3, 512], F32, name="bc", tag="sg")
                nc.tensor.matmul(
                    bc[0 : HD + 1, 0, 0:qw],
                    lhsT=ones_sb[:, 0 : HD + 1],
                    rhs=rA[:, 0:qw],
                    start=True, stop=True,
                )
                nc.tensor.matmul(
                    bc[64 : 64 + HD + 1, 1, 0:qw],
                    lhsT=ones_sb[:, 0 : HD + 1],
                    rhs=rB[:, 0:qw],
                    start=True, stop=True,
                )
                # stage broadcast in SBUF: DVE may read at most one PSUM operand
                bcs = rc_pool.tile([P, 512], F32, name="bcs", tag="bcs")
                nc.vector.tensor_copy(bcs[0 : HD + 1, 0:qw], bc[0 : HD + 1, 0, 0:qw])
                nc.vector.tensor_copy(
                    bcs[64 : 64 + HD + 1, 0:qw], bc[64 : 64 + HD + 1, 1, 0:qw]
                )
                nc.vector.tensor_mul(
                    oT_sb[0 : HD + 1, pr, q0 : q0 + qw],
                    oTA[0 : HD + 1, 0:qw],
                    bcs[0 : HD + 1, 0:qw],
                )
                nc.vector.tensor_mul(
                    oT_sb[64 : 64 + HD + 1, pr, q0 : q0 + qw],
                    oTB[64 : 64 + HD + 1, 0:qw],
                    bcs[64 : 64 + HD + 1, 0:qw],
                )

    # ---------------- output projection ---------------------------------
    with (
        tc.tile_pool(name="fin", bufs=3) as fin_pool,
        tc.tile_pool(name="psF", bufs=4, space="PSUM") as psF,
    ):
        for nt in range(NT):
            fA = psF.tile([P, C], F32, name="fA", tag="f")
            fB = psF.tile([P, C], F32, name="fB", tag="f")
            for pr in range(PAIRS):
                # K=49 including the denom row; pw row 0 / 64 is zero
                nc.tensor.matmul(
                    fA[:],
                    lhsT=oT_sb[0 : HD + 1, pr, nt * P : (nt + 1) * P],
                    rhs=pw_sb[0 : HD + 1, pr, :],
                    start=(pr == 0),
                    stop=False,
                )
                nc.tensor.matmul(
                    fB[:],
                    lhsT=oT_sb[64 : 64 + HD + 1, pr, nt * P : (nt + 1) * P],
                    rhs=pw_sb[64 : 64 + HD + 1, pr, :],
                    start=(pr == 0),
                    stop=(pr == PAIRS - 1),
                )
            nc.tensor.matmul(
                fA[:], lhsT=ones_sb[:, 0:P], rhs=pb_sb[:], start=False, stop=True
            )
            ft = fin_pool.tile([P, C], F32, name="ft", tag="ft")
            nc.vector.tensor_copy(ft[:], fA[:])
            nc.vector.tensor_add(ft[:], ft[:], fB[:])
            nc.sync.dma_start(d["out"][nt * P : (nt + 1) * P, :], ft[:])


def build_program(n_cores: int = 8):
    nc = bacc.Bacc(
        "TRN2",
        target_bir_lowering=False,
        debug=False,
        enable_asserts=False,
        num_devices=n_cores,
    )
    d = {
        "xT": nc.dram_tensor("xT", [C, N], MM_DT, kind="ExternalInput").ap(),
        "wqP": nc.dram_tensor("wqP", [C, PAIRS * P], MM_DT, kind="ExternalInput").ap(),
        "wkP": nc.dram_tensor("wkP", [C, PAIRS * P], MM_DT, kind="ExternalInput").ap(),
        "wvA": nc.dram_tensor("wvA", [C, VW], MM_DT, kind="ExternalInput").ap(),
        "vbA": nc.dram_tensor("vbA", [1, VW], MM_DT, kind="ExternalInput").ap(),
        "qbP": nc.dram_tensor("qbP", [P, PAIRS], F32, kind="ExternalInput").ap(),
        "kbP": nc.dram_tensor("kbP", [P, PAIRS], F32, kind="ExternalInput").ap(),
        "pwP": nc.dram_tensor("pwP", [PAIRS, P, C], MM_DT, kind="ExternalInput").ap(),
        "pbR": nc.dram_tensor("pbR", [1, C], MM_DT, kind="ExternalInput").ap(),
        "out": nc.dram_tensor("out", [N, C], F32, kind="ExternalOutput").ap(),
    }
    import contextlib

    with tile.TileContext(nc) as tc:
        with contextlib.ExitStack() as ctx:
            _emit(tc, d, ctx)
    nc.finalize()
    return nc


def _prep_host(x, q_w, q_b, kv_w, kv_b, proj_w, proj_b):
    """Transpose/pack on host. Returns (per-core xT list, shared map)."""
    f32 = np.float32
    x = np.asarray(x, f32)
    xT = np.ascontiguousarray(x.reshape(B, N, C).transpose(0, 2, 1))  # [B, C, N]

    qwT = np.ascontiguousarray(np.asarray(q_w, f32).T)  # [Cin, Cout]
    kwT = np.ascontiguousarray(np.asarray(kv_w[:C], f32).T)
    vwT = np.ascontiguousarray(np.asarray(kv_w[C:], f32).T)
    pwT = np.ascontiguousarray(np.asarray(proj_w, f32).T)

    wqP = np.zeros((C, PAIRS * P), f32)
    wkP = np.zeros((C, PAIRS * P), f32)
    qbP = np.zeros((P, PAIRS), f32)
    kbP = np.zeros((P, PAIRS), f32)
    pwP = np.zeros((PAIRS, P, C), f32)
    for p in range(PAIRS):
        a, b = 2 * p, 2 * p + 1
        wqP[:, p * P : p * P + HD] = qwT[:, a * HD : (a + 1) * HD]
        wqP[:, p * P + 64 : p * P + 64 + HD] = qwT[:, b * HD : (b + 1) * HD]
        wkP[:, p * P : p * P + HD] = kwT[:, a * HD : (a + 1) * HD]
        wkP[:, p * P + 64 : p * P + 64 + HD] = kwT[:, b * HD : (b + 1) * HD]
        qbP[0:HD, p] = q_b[a * HD : (a + 1) * HD]
        qbP[64 : 64 + HD, p] = q_b[b * HD : (b + 1) * HD]
        kbP[0:HD, p] = kv_b[a * HD : (a + 1) * HD]
        kbP[64 : 64 + HD, p] = kv_b[b * HD : (b + 1) * HD]
        # rows 1..48 / 65..112 carry the proj weights; rows 0 / 64 stay zero
        # to swallow the denominator row of outT.
        pwP[p, 1 : 1 + HD, :] = pwT[a * HD : (a + 1) * HD, :]
        pwP[p, 65 : 65 + HD, :] = pwT[b * HD : (b + 1) * HD, :]

    # V blocks are [ones | v0..v47] per head so the softmax denominator lands
    # at a 32-aligned PSUM partition (0 / 64).
    wvA = np.zeros((C, VW), f32)
    vbA = np.zeros((1, VW), f32)
    for h in range(NH):
        wvA[:, h * (HD + 1) + 1 : (h + 1) * (HD + 1)] = vwT[:, h * HD : (h + 1) * HD]
        vbA[0, h * (HD + 1) + 1 : (h + 1) * (HD + 1)] = kv_b[
            C + h * HD : C + (h + 1) * HD
        ]
        vbA[0, h * (HD + 1)] = 1.0

    shared = {
        "wqP": wqP,
        "wkP": wkP,
        "wvA": wvA,
        "vbA": vbA,
        "qbP": qbP,
        "kbP": kbP,
        "pwP": pwP,
        "pbR": np.asarray(proj_b, f32).reshape(1, C),
    }
    return xT, shared


_PROGRAM = None


def _get_program():
    global _PROGRAM
    if _PROGRAM is None:
        _PROGRAM = build_program(B)
    return _PROGRAM


def kernel(x, q_w, q_b, kv_w, kv_b, proj_w, proj_b):
    xT, shared = _prep_host(x, q_w, q_b, kv_w, kv_b, proj_w, proj_b)
    nc = _get_program()
    in_maps = [dict(shared, xT=np.ascontiguousarray(xT[b])) for b in range(B)]
    res = run_bass_kernel_spmd(nc, in_maps, list(range(B)))
    outs = [np.asarray(res.results[i]["out"], np.float32) for i in range(B)]
    return np.stack(outs).reshape(B, HH, WW, C)


# revision 13
# speedup vs baseline: 2.1587x; 2.1587x over previous
"""Trainium2 Bass kernel for nn_Attention_17532056502607.

Multi-head self-attention (B=8, N=48*48=2304 tokens, C=384, 8 heads of 48):
    q = x @ q_w.T + q_b ; k,v = x @ kv_w.T + kv_b
    out = softmax(q k^T / sqrt(48)) v ; y = out @ proj_w.T + proj_b

Sharding: data-parallel, one batch element per NeuronCore (8 cores).

Per-core algorithm (all in "S^T layout", keys on partitions — no transposes):
  - host supplies xT = x_b^T [C, N] and head-PAIR-packed weights: heads 2p /
    2p+1 of a pair live at partition rows 0-47 / 64-111, so two K=48 matmuls
    run concurrently in the PE array (row/col 32-tiles).
  - qT/kT [C_pair, N] = wT-pair @ xT          (PE, K=C=384)
  - v    [N, 8*49]    = xT-pair-cols... v natural + a ones column per head
    (ones column injected via a K=1 rank-1 matmul with the v-bias row), so
    attn@V also accumulates the softmax denominator.
  - S^T  [keys, q]    = kT-tile.T @ qT        (K=48, row-packed head pairs)
  - expS = exp(scale * S^T)                   (ACT, reads PSUM, writes SBUF)
  - outT [49*2, q]   += (v|1).T @ expS        (K=128 keys, col-packed pairs)
    row 48/112 of outT = softmax denominators.
  - normalize: recip(denoms) -> rank-1 ones-matmul broadcast -> DVE multiply
  - y    [N, C]       = sum_pairs outT-pair.T @ projw-pair + bias (row-packed)
"""

import os
import sys

import numpy as np

for _p in ("/opt/trn_rl_repo",):
    if _p not in sys.path:
        sys.path.append(_p)

import concourse.bass as bass  # noqa: E402
import concourse.tile as tile  # noqa: E402
from concourse import bacc, mybir  # noqa: E402
from concourse.bass_utils import run_bass_kernel_spmd  # noqa: E402

# ---------------------------------------------------------------- constants
B = 8
HH = 48
WW = 48
C = 384
N = HH * WW  # 2304
NH = 8
HD = 48
PAIRS = NH // 2  # 4
P = 128
NT = N // P  # 18 token tiles
KTC = C // P  # 3 contraction tiles over C
SCALE = float(HD) ** -0.5
VW = NH * (HD + 1)  # 392: v with a ones column per head
CHUNKS = [(0, 512), (512, 512), (1024, 512), (1536, 512), (2048, 256)]

F32 = mybir.dt.float32
# Matmul dtype for all SBUF operands. float32 = exact but 4 cyc/row on PE;
# float32r = same bits, reduced-precision single-pass matmul (1 cyc/row for
# moving dim >= 256) but cannot write PSUM at partition base 64; bfloat16
# halves SBUF/DMA and enables FWL.
MM_DT = getattr(mybir.dt, os.environ.get("ATTN_MM_DT", "float32r"))
# attn@V runs in bf16 when MM_DT is float32r (f32r matmuls cannot col-tile to
# partition base 64; bf16 error here is averaged over the 2304-key softmax).
AV_DT = (
    mybir.dt.bfloat16
    if MM_DT == mybir.dt.float32r
    else getattr(mybir.dt, os.environ.get("ATTN_AV_DT", MM_DT.value))
)

_EXP = mybir.ActivationFunctionType.Exp


def _emit(tc: tile.TileContext, d: dict, ctx):
    nc = tc.nc

    persist = ctx.enter_context(tc.tile_pool(name="persist", bufs=1))
    v_sb = persist.tile([P, NT, VW], AV_DT, name="v_sb")
    qT_sb = persist.tile([P, PAIRS, N], MM_DT, name="qT_sb")
    kT_sb = persist.tile([P, PAIRS, N], MM_DT, name="kT_sb")
    oT_sb = persist.tile([P, PAIRS, N], MM_DT, name="oT_sb")
    pw_sb = persist.tile([P, PAIRS, C], MM_DT, name="pw_sb")
    qb_sb = persist.tile([P, PAIRS], F32, name="qb_sb")
    kb_sb = persist.tile([P, PAIRS], F32, name="kb_sb")
    vb_sb = persist.tile([1, VW], MM_DT, name="vb_sb")
    pb_sb = persist.tile([1, C], MM_DT, name="pb_sb")
    # fp32 ones vector (memset can't encode float32r); bitcast where an
    # MM_DT-typed operand is required — the bit pattern is identical.
    ones32 = persist.tile([1, P], F32, name="ones32")

    nc.sync.dma_start(pw_sb[:], d["pwP"].rearrange("r p m -> p r m"))
    nc.sync.dma_start(qb_sb[:], d["qbP"])
    nc.sync.dma_start(kb_sb[:], d["kbP"])
    nc.sync.dma_start(vb_sb[:], d["vbA"])
    nc.sync.dma_start(pb_sb[:], d["pbR"])
    nc.vector.memset(ones32[:], 1.0)
    if MM_DT == mybir.dt.bfloat16:
        ones_mm = persist.tile([1, P], MM_DT, name="ones_mm")
        nc.vector.memset(ones_mm[:], 1.0)
    elif MM_DT == mybir.dt.float32r:
        ones_mm = ones32.bitcast(MM_DT)
    else:
        ones_mm = ones32

    # ---------------- phase A: projections q^T, k^T, v ----------------
    with (
        tc.tile_pool(name="phA", bufs=1) as phA,
        tc.tile_pool(name="psA", bufs=4, space="PSUM") as psA,
    ):
        xT_sb = phA.tile([P, KTC, N], MM_DT, name="xT_sb")
        wq_sb = phA.tile([P, KTC, PAIRS * P], MM_DT, name="wq_sb")
        wk_sb = phA.tile([P, KTC, PAIRS * P], MM_DT, name="wk_sb")
        wv_sb = phA.tile([P, KTC, VW], MM_DT, name="wv_sb")
        nc.sync.dma_start(xT_sb[:], d["xT"].rearrange("(kt p) n -> p kt n", p=P))
        nc.sync.dma_start(wq_sb[:], d["wqP"].rearrange("(kt p) m -> p kt m", p=P))
        nc.sync.dma_start(wk_sb[:], d["wkP"].rearrange("(kt p) m -> p kt m", p=P))
        nc.sync.dma_start(wv_sb[:], d["wvA"].rearrange("(kt p) m -> p kt m", p=P))

        # v natural [token, 8*(hd|1)]: K=C matmul + rank-1 (ones x vb_aug)
        # which adds the v bias AND writes 1.0 into each head's 49th column.
        for nt in range(NT):
            psv = psA.tile([P, 512], F32, name="psv", tag="psA")
            for kt in range(KTC):
                nc.tensor.matmul(
                    psv[:, 0:VW],
                    lhsT=xT_sb[:, kt, nt * P : (nt + 1) * P],
                    rhs=wv_sb[:, kt, :],
                    start=(kt == 0),
                    stop=False,
                )
            nc.tensor.matmul(
                psv[:, 0:VW],
                lhsT=ones_mm[:, 0:P],
                rhs=vb_sb[:],
                start=False,
                stop=True,
            )
            nc.vector.tensor_copy(v_sb[:, nt, :], psv[:, 0:VW])

        # q^T/k^T in pair layout: out partitions = pair block of C_out.
        for pr in range(PAIRS):
            for q0, qw in CHUNKS:
                psq = psA.tile([P, 512], F32, name="psq", tag="psA")
                psk = psA.tile([P, 512], F32, name="psk", tag="psA")
                for kt in range(KTC):
                    nc.tensor.matmul(
                        psq[:, 0:qw],
                        lhsT=wq_sb[:, kt, pr * P : (pr + 1) * P],
                        rhs=xT_sb[:, kt, q0 : q0 + qw],
                        start=(kt == 0),
                        stop=(kt == KTC - 1),
                    )
                    nc.tensor.matmul(
                        psk[:, 0:qw],
                        lhsT=wk_sb[:, kt, pr * P : (pr + 1) * P],
                        rhs=xT_sb[:, kt, q0 : q0 + qw],
                        start=(kt == 0),
                        stop=(kt == KTC - 1),
                    )
                nc.vector.tensor_scalar_add(
                    qT_sb[:, pr, q0 : q0 + qw], psq[:, 0:qw], qb_sb[:, pr : pr + 1]
                )
                nc.vector.tensor_scalar_add(
                    kT_sb[:, pr, q0 : q0 + qw], psk[:, 0:qw], kb_sb[:, pr : pr + 1]
                )

    # ---------------- attention: flash over q chunks, S^T layout -------
    with (
        tc.tile_pool(name="es", bufs=3) as es_pool,
        tc.tile_pool(name="rcp", bufs=4) as rc_pool,
        tc.tile_pool(name="psS", bufs=2, space="PSUM") as psS,
        tc.tile_pool(name="psO", bufs=1, space="PSUM") as psO,
    ):
        for pr in range(PAIRS):
            for q0, qw in CHUNKS:
                # separate accumulator banks per head; col strips 0-1 vs 2-3
                # (out base partition 0 vs 64) keep the two matmuls concurrent
                oTA = psO.tile([P, 512], F32, name="oTA", tag="oTA")
                oTB = psO.tile([P, 512], F32, name="oTB", tag="oTB")
                # slot sequence: si = 2*kt + head_bit, grouped 3 per psum tile
                seq = [(kt, hoff) for kt in range(NT) for hoff in (0, 64)]
                sg = None
                for si, (kt, hoff) in enumerate(seq):
                    g, j = divmod(si, 3)
                    if j == 0:
                        sg = psS.tile([P, 3, 512], F32, name="sg", tag="sg")
                    nc.tensor.matmul(
                        sg[:, j, 0:qw],
                        lhsT=kT_sb[hoff : hoff + HD, pr, kt * P : (kt + 1) * P],
                        rhs=qT_sb[hoff : hoff + HD, pr, q0 : q0 + qw],
                        start=True,
                        stop=True,
                    )
                    if j == 2:
                        est = es_pool.tile([P, 3, 512], AV_DT, name="est", tag="est")
                        nc.scalar.activation(
                            est[:, :, 0:qw], sg[:, :, 0:qw], _EXP, scale=SCALE
                        )
                        for jj in range(3):
                            kt2, hoff2 = seq[g * 3 + jj]
                            h = pr * 2 + (0 if hoff2 == 0 else 1)
                            oT = oTA if hoff2 == 0 else oTB
                            nc.tensor.matmul(
                                oT[hoff2 : hoff2 + HD + 1, 0:qw],
                                lhsT=v_sb[:, kt2, h * (HD + 1) : (h + 1) * (HD + 1)],
                                rhs=est[:, jj, 0:qw],
                                start=(kt2 == 0),
                                stop=(kt2 == NT - 1),
                            )
                # normalize: row 0 of oTA / row 64 of oTB = denominators
                # (ones column leads each V block so denoms land 32-aligned)
                rA = rc_pool.tile([1, 512], F32, name="rA", tag="rc")
                rB = rc_pool.tile([1, 512], F32, name="rB", tag="rc")
                nc.vector.reciprocal(rA[:, 0:qw], oTA[0:1, 0:qw])
                nc.vector.reciprocal(rB[:, 0:qw], oTB[64:65, 0:qw])
                bc = psS.tile([P, 3, 512], F32, name="bc", tag="sg")
                nc.tensor.matmul(
                    bc[0 : HD + 1, 0, 0:qw],
                    lhsT=ones32[:, 0 : HD + 1],
                    rhs=rA[:, 0:qw],
                    start=True, stop=True,
                )
                nc.tensor.matmul(
                    bc[64 : 64 + HD + 1, 1, 0:qw],
                    lhsT=ones32[:, 0 : HD + 1],
                    rhs=rB[:, 0:qw],
                    start=True, stop=True,
                )
                # stage broadcast in SBUF: DVE may read at most one PSUM operand
                bcs = rc_pool.tile([P, 512], F32, name="bcs", tag="bcs")
                nc.vector.tensor_copy(bcs[0 : HD + 1, 0:qw], bc[0 : HD + 1, 0, 0:qw])
                nc.vector.tensor_copy(
                    bcs[64 : 64 + HD + 1, 0:qw], bc[64 : 64 + HD + 1, 1, 0:qw]
                )
                nc.vector.tensor_mul(
                    oT_sb[0 : HD + 1, pr, q0 : q0 + qw],
                    oTA[0 : HD + 1, 0:qw],
                    bcs[0 : HD + 1, 0:qw],
                )
                nc.vector.tensor_mul(
                    oT_sb[64 : 64 + HD + 1, pr, q0 : q0 + qw],
                    oTB[64 : 64 + HD + 1, 0:qw],
                    bcs[64 : 64 + HD + 1, 0:qw],
                )

    # ---------------- output projection ---------------------------------
    with (
        tc.tile_pool(name="fin", bufs=3) as fin_pool,
        tc.tile_pool(name="psF", bufs=4, space="PSUM") as psF,
    ):
        for nt in range(NT):
            fA = psF.tile([P, C], F32, name="fA", tag="f")
            fB = psF.tile([P, C], F32, name="fB", tag="f")
            for pr in range(PAIRS):
                # K=49 including the denom row; pw row 0 / 64 is zero
                nc.tensor.matmul(
                    fA[:],
                    lhsT=oT_sb[0 : HD + 1, pr, nt * P : (nt + 1) * P],
                    rhs=pw_sb[0 : HD + 1, pr, :],
                    start=(pr == 0),
                    stop=False,
                )
                nc.tensor.matmul(
                    fB[:],
                    lhsT=oT_sb[64 : 64 + HD + 1, pr, nt * P : (nt + 1) * P],
                    rhs=pw_sb[64 : 64 + HD + 1, pr, :],
                    start=(pr == 0),
                    stop=(pr == PAIRS - 1),
                )
            nc.tensor.matmul(
                fA[:], lhsT=ones_mm[:, 0:P], rhs=pb_sb[:], start=False, stop=True
            )
            ft = fin_pool.tile([P, C], F32, name="ft", tag="ft")
            nc.vector.tensor_copy(ft[:], fA[:])
            nc.vector.tensor_add(ft[:], ft[:], fB[:])
            nc.sync.dma_start(d["out"][nt * P : (nt + 1) * P, :], ft[:])


def build_program(n_cores: int = 8):
    nc = bacc.Bacc(
        "TRN2",
        target_bir_lowering=False,
        debug=False,
        enable_asserts=False,
        num_devices=n_cores,
    )
    d = {
        "xT": nc.dram_tensor("xT", [C, N], MM_DT, kind="ExternalInput").ap(),
        "wqP": nc.dram_tensor("wqP", [C, PAIRS * P], MM_DT, kind="ExternalInput").ap(),
        "wkP": nc.dram_tensor("wkP", [C, PAIRS * P], MM_DT, kind="ExternalInput").ap(),
        "wvA": nc.dram_tensor("wvA", [C, VW], MM_DT, kind="ExternalInput").ap(),
        "vbA": nc.dram_tensor("vbA", [1, VW], MM_DT, kind="ExternalInput").ap(),
        "qbP": nc.dram_tensor("qbP", [P, PAIRS], F32, kind="ExternalInput").ap(),
        "kbP": nc.dram_tensor("kbP", [P, PAIRS], F32, kind="ExternalInput").ap(),
        "pwP": nc.dram_tensor("pwP", [PAIRS, P, C], MM_DT, kind="ExternalInput").ap(),
        "pbR": nc.dram_tensor("pbR", [1, C], MM_DT, kind="ExternalInput").ap(),
        "out": nc.dram_tensor("out", [N, C], F32, kind="ExternalOutput").ap(),
    }
    import contextlib

    with tile.TileContext(nc) as tc:
        with contextlib.ExitStack() as ctx:
            _emit(tc, d, ctx)
    nc.finalize()
    return nc


def _prep_host(x, q_w, q_b, kv_w, kv_b, proj_w, proj_b):
    """Transpose/pack on host. Returns (per-core xT list, shared map)."""
    f32 = np.float32
    x = np.asarray(x, f32)
    xT = np.ascontiguousarray(x.reshape(B, N, C).transpose(0, 2, 1))  # [B, C, N]

    qwT = np.ascontiguousarray(np.asarray(q_w, f32).T)  # [Cin, Cout]
    kwT = np.ascontiguousarray(np.asarray(kv_w[:C], f32).T)
    vwT = np.ascontiguousarray(np.asarray(kv_w[C:], f32).T)
    pwT = np.ascontiguousarray(np.asarray(proj_w, f32).T)

    wqP = np.zeros((C, PAIRS * P), f32)
    wkP = np.zeros((C, PAIRS * P), f32)
    qbP = np.zeros((P, PAIRS), f32)
    kbP = np.zeros((P, PAIRS), f32)
    pwP = np.zeros((PAIRS, P, C), f32)
    for p in range(PAIRS):
        a, b = 2 * p, 2 * p + 1
        wqP[:, p * P : p * P + HD] = qwT[:, a * HD : (a + 1) * HD]
        wqP[:, p * P + 64 : p * P + 64 + HD] = qwT[:, b * HD : (b + 1) * HD]
        wkP[:, p * P : p * P + HD] = kwT[:, a * HD : (a + 1) * HD]
        wkP[:, p * P + 64 : p * P + 64 + HD] = kwT[:, b * HD : (b + 1) * HD]
        qbP[0:HD, p] = q_b[a * HD : (a + 1) * HD]
        qbP[64 : 64 + HD, p] = q_b[b * HD : (b + 1) * HD]
        kbP[0:HD, p] = kv_b[a * HD : (a + 1) * HD]
        kbP[64 : 64 + HD, p] = kv_b[b * HD : (b + 1) * HD]
        # rows 1..48 / 65..112 carry the proj weights; rows 0 / 64 stay zero
        # to swallow the denominator row of outT.
        pwP[p, 1 : 1 + HD, :] = pwT[a * HD : (a + 1) * HD, :]
        pwP[p, 65 : 65 + HD, :] = pwT[b * HD : (b + 1) * HD, :]

    # V blocks are [ones | v0..v47] per head so the softmax denominator lands
    # at a 32-aligned PSUM partition (0 / 64).
    wvA = np.zeros((C, VW), f32)
    vbA = np.zeros((1, VW), f32)
    for h in range(NH):
        wvA[:, h * (HD + 1) + 1 : (h + 1) * (HD + 1)] = vwT[:, h * HD : (h + 1) * HD]
        vbA[0, h * (HD + 1) + 1 : (h + 1) * (HD + 1)] = kv_b[
            C + h * HD : C + (h + 1) * HD
        ]
        vbA[0, h * (HD + 1)] = 1.0

    shared = {
        "wqP": wqP,
        "wkP": wkP,
        "wvA": wvA,
        "vbA": vbA,
        "qbP": qbP,
        "kbP": kbP,
        "pwP": pwP,
        "pbR": np.asarray(proj_b, f32).reshape(1, C),
    }
    return xT, shared


_PROGRAM = None


def _get_program():
    global _PROGRAM
    if _PROGRAM is None:
        _PROGRAM = build_program(B)
    return _PROGRAM


def kernel(x, q_w, q_b, kv_w, kv_b, proj_w, proj_b):
    xT, shared = _prep_host(x, q_w, q_b, kv_w, kv_b, proj_w, proj_b)
    nc = _get_program()
    in_maps = [dict(shared, xT=np.ascontiguousarray(xT[b])) for b in range(B)]
    res = run_bass_kernel_spmd(nc, in_maps, list(range(B)))
    outs = [np.asarray(res.results[i]["out"], np.float32) for i in range(B)]
    return np.stack(outs).reshape(B, HH, WW, C)
